# revision 2
# baseline (speedup 1.0000x reference)
"""Trainium2 Bass kernel for nn_DWTModelSimple.

The reference computes a 2-level orthonormal Haar DWT and immediately
inverts it with the exact same cached high-frequency subbands.  Per 2x2
block the inverse butterfly reconstructs a,b,c,d exactly, so
idwt(idwt(dwt(dwt(x)))) == x: the whole module is the identity map.
The float32 reference deviates from x only by its own rounding noise
(~6e-8 norm-relative), so the memory-roofline implementation is a
straight HBM->HBM copy, data-parallel over the batch dimension
(batch 32 -> 4 per core across 8 NeuronCores; 12.58 MB per core).

The copy streams through both HWDGE rings (SP + ACT), i.e. 2 queues on
each of the 16 SDMA engines.  Profiling findings this build encodes:

* Descriptor i of a PDMA2D trigger is serviced by SDMA engine i%16,
  restarting at engine 0 for every trigger (verified from per-queue
  byte counts in the NTFF profile).
* SDMA engine 15 runs ~0.85x the rate of engines 0-14 and starts
  ~2 us late; at equal load it finished ~7 us after the pack, directly
  delaying each ring's completion semaphore.  Engines 11-14 start ~1 us
  late (they are last in descriptor round-robin order).  The layout
  below gives engine 15 ~0.80x and engine 14 ~0.94x of a regular
  engine's bytes, which made all engines finish within ~1 us.
* The AP optimizer merges contiguous rows and re-splits 16-wide, so
  sub-16-descriptor triggers must use stride-2 interleaved row pairs
  (two triggers of [R, stride 2] covering a 2R-row region) whose APs
  cannot be merged.  Contiguous base regions are left mergeable on
  purpose: they lower to 48 KB descriptors spread evenly 16-wide.
* HWDGE posts exactly 16 completion increments per trigger (one per
  SDMA engine, regardless of the trigger's descriptor count), so each
  ring waits for 16 x n_triggers on its own semaphore.
* The measured exec window starts at the first DMA trigger (the NEFF
  entry ABI before it is excluded) and ends at the last engine halt;
  the NRT-injected exit ABI (two $S[2] barriers + a 5-way-split clear
  sweep of semaphores 3..255 + NOTIFY) is a fixed ~7 us tail that
  load-time patching appends to the instruction streams - it is not in
  the walrus-emitted engine binaries and cannot be trimmed from the
  kernel side.

The module is built straight-line and then IR-spliced so the DMA
trigger instructions execute ahead of bass's init-barrier run (the
stream launches the moment the NEFF entry sequence ends).  A guarded
fallback rebuilds the plain Block form if the preamble structure ever
changes.

Measured on 8 axon trn2 cores (best of 3, NTFF profile of core 0):
baseline even-split build 53.3 us; this layout 48.2 us, with the
remaining time = 12.58 MB payload streaming at ~650 GB/s HBM
read+write plus the fixed ~7 us exit ABI.
"""

import numpy as np

import concourse.bass as bass
import concourse.mybir as mybir
from concourse.bass_utils import run_bass_kernel_spmd

N_CORES = 8
B, C, H, W = 32, 3, 512, 512
B_PER_CORE = B // N_CORES
ELEMS_PER_CORE = B_PER_CORE * C * H * W  # 3,145,728

QUANT = 6144                      # elems per row (24,576 B descriptors)
N_ROWS = ELEMS_PER_CORE // QUANT  # 512
P = N_ROWS
FREE = QUANT

# Per-ring layout: contiguous base regions (rows; keep multiples of 32 so
# they lower to evenly-spread 48 KB descriptors), stride-2 trim pairs
# (r1, r2) with r2 in {r1, r1-1} covering r1+r2 rows and loading only
# engines 0..r1-1 / 0..r2-1, and a remainder region (spread 16-wide as
# small descriptors).
RING_CFG = (
    dict(base=(64, 64, 64), pairs=((15, 15), (15, 14)), rem=5),  # SP
    dict(base=(64, 64, 64), pairs=((15, 15), (15, 14)), rem=5),  # ACT
)
assert (
    sum(
        sum(c["base"]) + sum(r1 + r2 for r1, r2 in c["pairs"]) + c["rem"]
        for c in RING_CFG
    )
    == N_ROWS
)

_cached_nc = None


def _emit_ring(eng, sem, x, y, r0: int, cfg) -> tuple[int, int]:
    """Emit one ring's triggers starting at row r0. Returns (next_row,
    n_triggers)."""
    n = 0
    for rows in cfg["base"]:
        eng.dma_start(y[r0 : r0 + rows, :], x[r0 : r0 + rows, :]).then_inc(sem, 16)
        r0 += rows
        n += 1
    for r1, r2 in cfg["pairs"]:
        assert r1 - 1 <= r2 <= r1
        eng.dma_start(
            y[r0 : r0 + 2 * r1 : 2, :], x[r0 : r0 + 2 * r1 : 2, :]
        ).then_inc(sem, 16)
        eng.dma_start(
            y[r0 + 1 : r0 + 2 * r2 : 2, :], x[r0 + 1 : r0 + 2 * r2 : 2, :]
        ).then_inc(sem, 16)
        r0 += r1 + r2
        n += 2
    rem = cfg["rem"]
    if rem:
        eng.dma_start(y[r0 : r0 + rem, :], x[r0 : r0 + rem, :]).then_inc(sem, 16)
        r0 += rem
        n += 1
    return r0, n


def _build_nc_spliced() -> bass.Bass:
    """Straight-line build + IR splice: hoist the DMA trigger instructions
    ahead of bass's init-barrier run so the stream launches as soon as the
    NEFF entry sequence finishes.  The completion waits stay at the end of
    each engine's stream."""
    SP = mybir.EngineType.SP
    ACT = mybir.EngineType.Activation

    nc = bass.Bass()
    main = nc.m.functions[0].blocks[0]
    assert main.name == "main", main.name
    pre_n = len(main.instructions)

    x = nc.dram_tensor("x", [P, FREE], mybir.dt.float32, kind="ExternalInput")
    y = nc.dram_tensor("y", [P, FREE], mybir.dt.float32, kind="ExternalOutput")

    with nc.semaphore("sem_sp") as sem_sp, nc.semaphore("sem_act") as sem_act:
        r0, n_sp = _emit_ring(nc.sync, sem_sp, x, y, 0, RING_CFG[0])
        r0, n_act = _emit_ring(nc.scalar, sem_act, x, y, r0, RING_CFG[1])
        assert r0 == N_ROWS, r0
        # waits emitted last so the splice below can separate them
        nc.sync.wait_ge(sem_sp, 16 * n_sp)
        nc.scalar.wait_ge(sem_act, 16 * n_act)

    insts = main.instructions
    pre, user = list(insts[:pre_n]), list(insts[pre_n:])
    assert all(i.engine in (SP, ACT) for i in user)

    def split_engine(eng):
        mine = [i for i in user if i.engine == eng]
        waits = [i for i in mine if isinstance(i, mybir.InstEventSemaphore)]
        assert len(waits) == 1, [type(i).__name__ for i in mine]
        return [i for i in mine if i is not waits[0]], waits[0]

    sp_trig, sp_wait = split_engine(SP)
    act_trig, act_wait = split_engine(ACT)

    def splice_point(eng):
        # index of the first instruction of the engine's trailing
        # Drain/EventSemaphore run (the init barrier) in the preamble
        idxs = [k for k, i in enumerate(pre) if i.engine == eng]
        assert idxs
        j = len(idxs)
        while j > 0 and isinstance(
            pre[idxs[j - 1]], (mybir.InstDrain, mybir.InstEventSemaphore)
        ):
            j -= 1
        assert j < len(idxs), "no barrier run found"
        return idxs[j]

    p_sp = splice_point(SP)
    p_act = splice_point(ACT)
    new = []
    for k, inst in enumerate(pre):
        if k == p_sp:
            new.extend(sp_trig)
        if k == p_act:
            new.extend(act_trig)
        new.append(inst)
    new.append(sp_wait)
    new.append(act_wait)
    assert len(new) == len(insts), (len(new), len(insts))
    insts[:] = new
    return nc


def _build_nc_plain() -> bass.Bass:
    nc = bass.Bass()
    x = nc.dram_tensor("x", [P, FREE], mybir.dt.float32, kind="ExternalInput")
    y = nc.dram_tensor("y", [P, FREE], mybir.dt.float32, kind="ExternalOutput")

    with (
        nc.semaphore("sem_sp") as sem_sp,
        nc.semaphore("sem_act") as sem_act,
        nc.Block() as block,
    ):
        cfg_sp = RING_CFG[0]
        slab_sp = (
            sum(cfg_sp["base"])
            + sum(r1 + r2 for r1, r2 in cfg_sp["pairs"])
            + cfg_sp["rem"]
        )

        @block.sync
        def _(sync):
            _, n = _emit_ring(sync, sem_sp, x, y, 0, RING_CFG[0])
            sync.wait_ge(sem_sp, 16 * n)

        @block.scalar
        def _(scalar):
            _, n = _emit_ring(scalar, sem_act, x, y, slab_sp, RING_CFG[1])
            scalar.wait_ge(sem_act, 16 * n)

    return nc


def _build_nc() -> bass.Bass:
    try:
        return _build_nc_spliced()
    except Exception:
        # Fall back to the long-validated Block form if the preamble
        # structure ever changes under the splice's assertions.
        return _build_nc_plain()


def get_nc() -> bass.Bass:
    global _cached_nc
    if _cached_nc is None:
        _cached_nc = _build_nc()
    return _cached_nc


def kernel(x: np.ndarray) -> np.ndarray:
    x = np.ascontiguousarray(x, dtype=np.float32)
    assert x.shape == (B, C, H, W), x.shape

    in_maps = [
        {"x": x[i * B_PER_CORE : (i + 1) * B_PER_CORE].reshape(P, FREE)}
        for i in range(N_CORES)
    ]
    try:
        res = run_bass_kernel_spmd(get_nc(), in_maps, core_ids=list(range(N_CORES)))
    except Exception:
        # One retry for transient runtime hiccups (e.g. a core recovering
        # from a previous process's interrupted run).
        res = run_bass_kernel_spmd(get_nc(), in_maps, core_ids=list(range(N_CORES)))
    return np.concatenate(
        [res.results[i]["y"].reshape(B_PER_CORE, C, H, W) for i in range(N_CORES)],
        axis=0,
    )


# revision 4
# speedup vs baseline: 1.0131x; 1.0131x over previous
"""Trainium2 Bass kernel for nn_DWTModelSimple.

The reference computes a 2-level orthonormal Haar DWT and immediately
inverts it with the exact same cached high-frequency subbands.  Per 2x2
block the inverse butterfly reconstructs a,b,c,d exactly, so
idwt(idwt(dwt(dwt(x)))) == x: the whole module is the identity map.
The float32 reference deviates from x only by its own rounding noise
(~6e-8 norm-relative), so the memory-roofline implementation is a
straight HBM->HBM copy, data-parallel over the batch dimension
(batch 32 -> 4 per core across 8 NeuronCores; 12.58 MB per core).

The copy streams through both HWDGE rings (SP + ACT), i.e. 2 queues on
each of the 16 SDMA engines.  Profiling findings this build encodes:

* Descriptor i of a PDMA2D trigger is serviced by SDMA engine i%16,
  restarting at engine 0 for every trigger (verified from per-queue
  byte counts in the NTFF profile).
* SDMA engine 15 runs ~0.85x the rate of engines 0-14 and starts
  ~2 us late; at equal load it finished ~7 us after the pack, directly
  delaying each ring's completion semaphore.  Engines 11-14 start ~1 us
  late (they are last in descriptor round-robin order).  The layout
  below gives engine 15 ~0.80x and engine 14 ~0.94x of a regular
  engine's bytes, which made all engines finish within ~1 us.
* The AP optimizer merges contiguous rows and re-splits 16-wide, so
  sub-16-descriptor triggers must use stride-2 interleaved row pairs
  (two triggers of [R, stride 2] covering a 2R-row region) whose APs
  cannot be merged.  Contiguous base regions are left mergeable on
  purpose: they lower to 48 KB descriptors spread evenly 16-wide.
* HWDGE posts exactly 16 completion increments per trigger (one per
  SDMA engine, regardless of the trigger's descriptor count), so each
  ring waits for 16 x n_triggers on its own semaphore.
* The measured exec window starts at the first DMA trigger (the NEFF
  entry ABI before it is excluded) and ends at the last engine halt;
  the NRT-injected exit ABI (two $S[2] barriers + a 5-way-split clear
  sweep of semaphores 3..255 + NOTIFY) is a fixed ~7 us tail that
  load-time patching appends to the instruction streams - it is not in
  the walrus-emitted engine binaries and cannot be trimmed from the
  kernel side.

The module is built straight-line and then IR-spliced so the DMA
trigger instructions execute ahead of bass's init-barrier run (the
stream launches the moment the NEFF entry sequence ends).  A guarded
fallback rebuilds the plain Block form if the preamble structure ever
changes.

Measured on 8 axon trn2 cores (best of 3, NTFF profile of core 0):
baseline even-split build 53.3 us; this layout 48.2 us, with the
remaining time = 12.58 MB payload streaming at ~650 GB/s HBM
read+write plus the fixed ~7 us exit ABI.
"""

import numpy as np

import concourse.bass as bass
import concourse.mybir as mybir
from concourse.bass_utils import run_bass_kernel_spmd

N_CORES = 8
B, C, H, W = 32, 3, 512, 512
B_PER_CORE = B // N_CORES
ELEMS_PER_CORE = B_PER_CORE * C * H * W  # 3,145,728

QUANT = 6144                      # elems per row (24,576 B descriptors)
N_ROWS = ELEMS_PER_CORE // QUANT  # 512
P = N_ROWS
FREE = QUANT

# Per-ring layout: contiguous base regions (rows; keep multiples of 32 so
# they lower to evenly-spread 48 KB descriptors), stride-2 trim pairs
# (r1, r2) with r2 in {r1, r1-1} covering r1+r2 rows and loading only
# engines 0..r1-1 / 0..r2-1, and a remainder region (spread 16-wide as
# small descriptors).
RING_CFG = (
    dict(base=(64, 64, 64), pairs=((16, 15), (15, 14)), rem=4),  # SP
    dict(base=(64, 64, 64), pairs=((16, 15), (15, 14)), rem=4),  # ACT
)
assert (
    sum(
        sum(c["base"]) + sum(r1 + r2 for r1, r2 in c["pairs"]) + c["rem"]
        for c in RING_CFG
    )
    == N_ROWS
)

_cached_nc = None


def _emit_ring(eng, sem, x, y, r0: int, cfg) -> tuple[int, int]:
    """Emit one ring's triggers starting at row r0. Returns (next_row,
    n_triggers)."""
    n = 0
    for rows in cfg["base"]:
        eng.dma_start(y[r0 : r0 + rows, :], x[r0 : r0 + rows, :]).then_inc(sem, 16)
        r0 += rows
        n += 1
    for r1, r2 in cfg["pairs"]:
        assert r1 - 1 <= r2 <= r1
        eng.dma_start(
            y[r0 : r0 + 2 * r1 - 1 : 2, :], x[r0 : r0 + 2 * r1 - 1 : 2, :]
        ).then_inc(sem, 16)
        eng.dma_start(
            y[r0 + 1 : r0 + 2 * r2 : 2, :], x[r0 + 1 : r0 + 2 * r2 : 2, :]
        ).then_inc(sem, 16)
        r0 += r1 + r2
        n += 2
    rem = cfg["rem"]
    if rem:
        eng.dma_start(y[r0 : r0 + rem, :], x[r0 : r0 + rem, :]).then_inc(sem, 16)
        r0 += rem
        n += 1
    return r0, n


def _build_nc_spliced() -> bass.Bass:
    """Straight-line build + IR splice: hoist the DMA trigger instructions
    ahead of bass's init-barrier run so the stream launches as soon as the
    NEFF entry sequence finishes.  The completion waits stay at the end of
    each engine's stream."""
    SP = mybir.EngineType.SP
    ACT = mybir.EngineType.Activation

    nc = bass.Bass()
    main = nc.m.functions[0].blocks[0]
    assert main.name == "main", main.name
    pre_n = len(main.instructions)

    x = nc.dram_tensor("x", [P, FREE], mybir.dt.float32, kind="ExternalInput")
    y = nc.dram_tensor("y", [P, FREE], mybir.dt.float32, kind="ExternalOutput")

    with nc.semaphore("sem_sp") as sem_sp, nc.semaphore("sem_act") as sem_act:
        r0, n_sp = _emit_ring(nc.sync, sem_sp, x, y, 0, RING_CFG[0])
        r0, n_act = _emit_ring(nc.scalar, sem_act, x, y, r0, RING_CFG[1])
        assert r0 == N_ROWS, r0
        # waits emitted last so the splice below can separate them
        nc.sync.wait_ge(sem_sp, 16 * n_sp)
        nc.scalar.wait_ge(sem_act, 16 * n_act)

    insts = main.instructions
    pre, user = list(insts[:pre_n]), list(insts[pre_n:])
    assert all(i.engine in (SP, ACT) for i in user)

    def split_engine(eng):
        mine = [i for i in user if i.engine == eng]
        waits = [i for i in mine if isinstance(i, mybir.InstEventSemaphore)]
        assert len(waits) == 1, [type(i).__name__ for i in mine]
        return [i for i in mine if i is not waits[0]], waits[0]

    sp_trig, sp_wait = split_engine(SP)
    act_trig, act_wait = split_engine(ACT)

    def splice_point(eng):
        # index of the first instruction of the engine's trailing
        # Drain/EventSemaphore run (the init barrier) in the preamble
        idxs = [k for k, i in enumerate(pre) if i.engine == eng]
        assert idxs
        j = len(idxs)
        while j > 0 and isinstance(
            pre[idxs[j - 1]], (mybir.InstDrain, mybir.InstEventSemaphore)
        ):
            j -= 1
        assert j < len(idxs), "no barrier run found"
        return idxs[j]

    p_sp = splice_point(SP)
    p_act = splice_point(ACT)
    new = []
    for k, inst in enumerate(pre):
        if k == p_sp:
            new.extend(sp_trig)
        if k == p_act:
            new.extend(act_trig)
        new.append(inst)
    new.append(sp_wait)
    new.append(act_wait)
    assert len(new) == len(insts), (len(new), len(insts))
    insts[:] = new
    return nc


def _build_nc_plain() -> bass.Bass:
    nc = bass.Bass()
    x = nc.dram_tensor("x", [P, FREE], mybir.dt.float32, kind="ExternalInput")
    y = nc.dram_tensor("y", [P, FREE], mybir.dt.float32, kind="ExternalOutput")

    with (
        nc.semaphore("sem_sp") as sem_sp,
        nc.semaphore("sem_act") as sem_act,
        nc.Block() as block,
    ):
        cfg_sp = RING_CFG[0]
        slab_sp = (
            sum(cfg_sp["base"])
            + sum(r1 + r2 for r1, r2 in cfg_sp["pairs"])
            + cfg_sp["rem"]
        )

        @block.sync
        def _(sync):
            _, n = _emit_ring(sync, sem_sp, x, y, 0, RING_CFG[0])
            sync.wait_ge(sem_sp, 16 * n)

        @block.scalar
        def _(scalar):
            _, n = _emit_ring(scalar, sem_act, x, y, slab_sp, RING_CFG[1])
            scalar.wait_ge(sem_act, 16 * n)

    return nc


def _build_nc() -> bass.Bass:
    try:
        return _build_nc_spliced()
    except Exception:
        # Fall back to the long-validated Block form if the preamble
        # structure ever changes under the splice's assertions.
        return _build_nc_plain()


def get_nc() -> bass.Bass:
    global _cached_nc
    if _cached_nc is None:
        _cached_nc = _build_nc()
    return _cached_nc


def kernel(x: np.ndarray) -> np.ndarray:
    x = np.ascontiguousarray(x, dtype=np.float32)
    assert x.shape == (B, C, H, W), x.shape

    in_maps = [
        {"x": x[i * B_PER_CORE : (i + 1) * B_PER_CORE].reshape(P, FREE)}
        for i in range(N_CORES)
    ]
    try:
        res = run_bass_kernel_spmd(get_nc(), in_maps, core_ids=list(range(N_CORES)))
    except Exception:
        # One retry for transient runtime hiccups (e.g. a core recovering
        # from a previous process's interrupted run).
        res = run_bass_kernel_spmd(get_nc(), in_maps, core_ids=list(range(N_CORES)))
    return np.concatenate(
        [res.results[i]["y"].reshape(B_PER_CORE, C, H, W) for i in range(N_CORES)],
        axis=0,
    )
